# revision 1
# baseline (speedup 1.0000x reference)
"""DenseFlashAttention (GNN segment-softmax attention) on 8 trn2 NeuronCores.

Sharding: receivers (and their incident edges) sharded across 8 cores;
DxD weights folded/replicated. Each core computes out rows for its 12500
receivers; host gathers.

Device algorithm per core:
  - receivers bin-packed into 392 groups of 32: 72 "big" groups with 768
    edge slots (6 tiles) absorb the heavy tail, 320 "small" groups with
    512 slots (4 tiles); per-group edge lists padded with self-masking
    pad edges.
  - stream xs_aug [67, ECAP]: rows 0..63 = x[sender]^T, 64 = 1,
    65 = rl (receiver slot in group; 32 for pads), 66 = rl^2.
  - per 128-edge tile, one fp32r PE matmul (256-wide rhs window for the
    1-cycle/row fp32r mode) vs per-group table [67, 97] inside the window:
      cols 0..63  -> Ve = x_s @ Wv
      col  64     -> 1.0
      cols 65..96 -> S[e,r] = a_r . x_s - C*(r - rl)^2,  a_r = A^T x_r,
                     A = Wq Wk^T * D^-0.5,  C = 64  (exact mask-in-matmul)
  - ACT exp -> P (p-valued segment indicator); PE matmul with
    lhsT=[Ve|1] accumulates [num|denom]^T per group in PSUM (out free=32).
  - final: out = x_loc + (num @ Wo) / denom  (division commutes past Wo).
"""

import os
import time
from contextlib import ExitStack

import numpy as np

# ---------------- static problem/config constants (hardcoded) ----------------
N = 100000
D = 64
E = 1600000
NCORES = 8
NLOC = N // NCORES            # 12500 receivers per core
RG = 32                       # receivers per group
NG = 392                      # groups per core
SLOTS = NG * RG               # 12544 receiver slots
NBIG = 72                     # 6-tile groups (cap 768 edges)
NSMALL = NG - NBIG            # 4-tile groups (cap 512 edges)
GT_BIG = 6
GT_SMALL = 4
GB_EDGE = GT_BIG * 128        # 768
GS_EDGE = GT_SMALL * 128      # 512
ECAP = NBIG * GB_EDGE + NSMALL * GS_EDGE   # 219136 edge slots per core
ROWS = 67                     # 64 x rows + ones + rl + rl^2
TBLW = 65 + RG                # 97 table cols per group
CMASK = 64.0                  # mask penalty coefficient
CH_G = 8                      # groups per DMA chunk
PSO_G = 4                     # groups per psum-out stack
NCHUNK = SLOTS // 128         # 98 final projection chunks
SCALE = D ** -0.5
MMW = 256                     # fp32r matmul window width (>=256 -> 1 cyc/row)
TSTR = 104                    # 32B-aligned psum window stride

_F32 = np.float32


def _group_base(g):
    return g * GB_EDGE if g < NBIG else \
        NBIG * GB_EDGE + (g - NBIG) * GS_EDGE


# ---------------- device kernel (built/compiled once) ----------------
_CACHE = {}


def _build_nc():
    import concourse.tile as tile
    from concourse import bacc, mybir

    f32 = mybir.dt.float32
    f32r = mybir.dt.float32r
    nc = bacc.Bacc("TRN2", target_bir_lowering=False, debug=False,
                   num_devices=NCORES)
    xs_ap = nc.dram_tensor("xs", [ROWS, ECAP], f32r,
                           kind="ExternalInput").ap()
    tbl_ap = nc.dram_tensor("tbl", [ROWS, (NG + 2) * TBLW], f32r,
                            kind="ExternalInput").ap()
    wo_ap = nc.dram_tensor("wo", [D, D], f32, kind="ExternalInput").ap()
    xloc_ap = nc.dram_tensor("xloc", [128, NCHUNK * D], f32,
                             kind="ExternalInput").ap()
    out_ap = nc.dram_tensor("out", [128, NCHUNK * D], f32,
                            kind="ExternalOutput").ap()

    EXP = mybir.ActivationFunctionType.Exp
    COPY = mybir.ActivationFunctionType.Copy

    with tile.TileContext(nc) as tc:
        with ExitStack() as octx:
            const_pool = octx.enter_context(tc.tile_pool(name="const", bufs=1))
            wo_sb = const_pool.tile([D, D], f32)
            nc.sync.dma_start(wo_sb[:], wo_ap[:, :])
            one_sb = const_pool.tile([65, 1], f32)
            nc.vector.memset(one_sb[:], 1.0)
            # rows 0..63: num^T staging; row 64: denom row
            odn_sb = const_pool.tile([65, SLOTS], f32)

            with ExitStack() as mctx:
                stream_pool = mctx.enter_context(
                    tc.tile_pool(name="stream", bufs=4))
                work_pool = mctx.enter_context(
                    tc.tile_pool(name="work", bufs=5))
                psA = mctx.enter_context(
                    tc.tile_pool(name="psA", bufs=3, space="PSUM"))
                psB = mctx.enter_context(
                    tc.tile_pool(name="psB", bufs=3, space="PSUM"))
                psO = mctx.enter_context(
                    tc.tile_pool(name="psO", bufs=2, space="PSUM"))
                FB = 7            # psum batches per final batch (98 = 14*7)

                def emit_final(fb):
                    # out rows for chunks k0..k0+FB-1: (num @ Wo)/denom + x
                    k0 = fb * FB
                    xt8 = fin_sb.tile([128, FB * D], f32, tag="xt")
                    nc.scalar.dma_start(xt8[:],
                                        xloc_ap[:, k0 * D:(k0 + FB) * D])
                    ps_d = fin_ps.tile([128, FB * 8], f32, tag="psd")
                    for j in range(FB):
                        nc.tensor.matmul(
                            out=ps_d[:, j * 8:j * 8 + 1],
                            lhsT=odn_sb[64:65,
                                        (k0 + j) * 128:(k0 + j + 1) * 128],
                            rhs=one_sb[64:65, :], start=True, stop=True)
                    dn8 = fin_sb.tile([128, FB], f32, tag="dn")
                    nc.vector.tensor_scalar_add(
                        dn8[:].rearrange("p (c o) -> p c o", o=1),
                        ps_d[:].rearrange(
                            "p (c o) -> p c o", o=8)[:, :, 0:1],
                        1e-30)
                    rec8 = fin_sb.tile([128, FB], f32, tag="rec")
                    nc.vector.reciprocal(rec8[:], dn8[:])
                    out8 = fin_sb.tile([128, FB * D], f32, tag="out8")
                    for j in range(FB):
                        k = k0 + j
                        ps_f = fin_ps.tile([128, D], f32, tag="psf")
                        nc.tensor.matmul(
                            out=ps_f[:],
                            lhsT=odn_sb[0:64, k * 128:(k + 1) * 128],
                            rhs=wo_sb[:], start=True, stop=True)
                        sc = fin_sb.tile([128, D], f32, tag="sc")
                        if j % 2 == 0:
                            nc.scalar.activation(
                                sc[:], ps_f[:],
                                COPY, scale=rec8[:, j:j + 1])
                        else:
                            nc.vector.tensor_scalar_mul(
                                sc[:], ps_f[:], rec8[:, j:j + 1])
                        nc.gpsimd.tensor_add(
                            out8[:, j * D:(j + 1) * D],
                            xt8[:, j * D:(j + 1) * D], sc[:])
                    nc.sync.dma_start(
                        out_ap[:, k0 * D:(k0 + FB) * D], out8[:])

                # software-pipelined: the [num|denom]^T matmuls for group
                # g-2 are emitted after the main matmuls of group g, so PE
                # never waits on the exp/copy of the current group.
                pending = []      # (p_all, veones, ps_outT, pgi, gt)

                def drain_one():
                    p_all, veones, pso, pgi, gt = pending.pop(0)
                    for t in range(gt):
                        nc.tensor.matmul(
                            out=pso[:, pgi * RG:(pgi + 1) * RG],
                            lhsT=veones[:, t * 65:(t + 1) * 65],
                            rhs=p_all[:, t * RG:(t + 1) * RG],
                            start=(t == 0), stop=(t == gt - 1))
                    if pgi == PSO_G - 1:
                        pbd = done_pb.pop(0)
                        nc.scalar.copy(
                            odn_sb[:, pbd * 128:(pbd + 1) * 128], pso[:])

                done_pb = []
                ps_outT = None
                chunks = ([(0, GT_BIG, 1), (1, GT_BIG, CH_G - 1)] +
                          [(cg * CH_G, GT_BIG, CH_G)
                           for cg in range(1, NBIG // CH_G)] +
                          [(NBIG + cg * CH_G, GT_SMALL, CH_G)
                           for cg in range(NSMALL // CH_G)])
                for g0, gt, ng in chunks:
                    gedge = gt * 128
                    xs_chunk = stream_pool.tile(
                        [ROWS, CH_G * GB_EDGE], f32r, tag="xs")
                    c0 = _group_base(g0)
                    nc.sync.dma_start(
                        xs_chunk[:, :ng * gedge],
                        xs_ap[:, c0:c0 + ng * gedge])
                    tbl_chunk = stream_pool.tile(
                        [ROWS, CH_G * TBLW + TBLW * 2], f32r, tag="tbl")
                    tc0 = g0 * TBLW
                    nc.sync.dma_start(
                        tbl_chunk[:, :(ng + 2) * TBLW],
                        tbl_ap[:, tc0:tc0 + (ng + 2) * TBLW])
                    for gi in range(ng):
                        g = g0 + gi
                        pb, pgi = divmod(g, PSO_G)
                        if pgi == 0:
                            # [num|denom]^T for 4 groups: [65, 4*32]
                            ps_outT = psO.tile([65, PSO_G * RG], f32,
                                               tag="pso")
                            done_pb.append(pb)
                        p_all = work_pool.tile([128, GT_BIG * RG], f32,
                                               tag="pall")
                        veones = work_pool.tile([128, GT_BIG * 65], f32,
                                                tag="veo")
                        half = gt // 2 + gt % 2
                        for pool, tag, t0, nt in ((psA, "ca", 0, half),
                                                  (psB, "cb", half,
                                                   gt - half)):
                            combo = pool.tile([128, 512], f32, tag=tag)
                            for ti in range(nt):
                                t = t0 + ti
                                lhsT = xs_chunk[:, gi * gedge + t * 128:
                                                gi * gedge + (t + 1) * 128]
                                nc.tensor.matmul(
                                    out=combo[:, ti * TSTR:ti * TSTR + MMW],
                                    lhsT=lhsT,
                                    rhs=tbl_chunk[:,
                                                  gi * TBLW:gi * TBLW + MMW],
                                    start=True, stop=True)
                            sub3 = combo[:, :nt * TSTR].rearrange(
                                "p (t c) -> p t c", c=TSTR)
                            nc.scalar.activation(
                                p_all[:, t0 * RG:(t0 + nt) * RG].rearrange(
                                    "p (t c) -> p t c", c=RG),
                                sub3[:, :, 65:TBLW], EXP)
                            nc.vector.tensor_copy(
                                veones[:, t0 * 65:(t0 + nt) * 65].rearrange(
                                    "p (t c) -> p t c", c=65),
                                sub3[:, :, 0:65])
                        pending.append((p_all, veones, ps_outT, pgi, gt))
                        if len(pending) > 3:
                            drain_one()
                while pending:
                    drain_one()

            with ExitStack() as fctx:
                fin_ps = fctx.enter_context(
                    tc.tile_pool(name="finps", bufs=3, space="PSUM"))
                fin_sb = fctx.enter_context(
                    tc.tile_pool(name="finsb", bufs=4))
                for fb in range(NCHUNK // FB):
                    emit_final(fb)

    nc.compile()
    return nc


def _get_nc():
    if "nc" not in _CACHE:
        t0 = time.time()
        _CACHE["nc"] = _build_nc()
        print(f"[kernel] bass trace+compile: {time.time()-t0:.1f}s",
              flush=True)
    return _CACHE["nc"]


# ---------------- host-side sharding / preprocessing ----------------

def _pack_groups(deg):
    """Assign receivers to groups: 72 big (<=768 edges) + 320 small
    (<=512), 32 receivers each. Returns (grp_of, slot_of)."""
    order = np.argsort(-deg, kind="stable")
    bin_of = np.empty(NLOC, np.int64)
    slot_of = np.empty(NLOC, np.int64)
    idx = np.arange(NLOC)
    bin_of[order] = idx % NG
    slot_of[order] = idx // NG
    bsum = np.bincount(bin_of, weights=deg.astype(np.float64), minlength=NG)
    # biggest-sum bins become the big class (groups 0..NBIG-1)
    rank = np.argsort(-bsum, kind="stable")
    perm = np.empty(NG, np.int64)
    perm[rank] = np.arange(NG)
    grp_of = perm[bin_of]
    gsum = np.bincount(grp_of, weights=deg.astype(np.float64), minlength=NG)

    members = [list(np.where(grp_of == g)[0]) for g in range(NG)]
    it = 0
    while True:
        small_over = [g for g in range(NBIG, NG) if gsum[g] > GS_EDGE]
        big_over = [g for g in range(NBIG) if gsum[g] > GB_EDGE]
        if not small_over and not big_over:
            break
        if small_over:
            gs = small_over[0]
            gb = int(np.argmin(gsum[:NBIG]))
            hs = max(members[gs], key=lambda r: deg[r])
            lb = min(members[gb], key=lambda r: deg[r])
        else:
            gb = big_over[0]
            gs = NBIG + int(np.argmin(gsum[NBIG:]))
            hs = min(members[gs], key=lambda r: deg[r])
            lb = max(members[gb], key=lambda r: deg[r])
        members[gs].remove(hs)
        members[gb].remove(lb)
        members[gs].append(lb)
        members[gb].append(hs)
        grp_of[hs], grp_of[lb] = gb, gs
        slot_of[hs], slot_of[lb] = slot_of[lb], slot_of[hs]
        gsum[gs] += deg[lb] - deg[hs]
        gsum[gb] += deg[hs] - deg[lb]
        it += 1
        assert it < 20000, "bin-pack repair failed"
    return grp_of, slot_of


def _prep_core(x, sender, receiver, A, Wv, core):
    """Build xs/tbl/xloc arrays + slot map for one core."""
    lo = core * NLOC
    mask = (receiver >= lo) & (receiver < lo + NLOC)
    snd = sender[mask]
    rcv = receiver[mask] - lo

    deg = np.bincount(rcv, minlength=NLOC)
    grp_of, slot_of = _pack_groups(deg)

    egrp = grp_of[rcv]
    eorder = np.argsort(egrp, kind="stable")
    cnt = np.bincount(egrp, minlength=NG)
    cum = np.concatenate([[0], np.cumsum(cnt)[:-1]])
    ofs = np.arange(len(eorder)) - np.repeat(cum, cnt)
    gbase = np.array([_group_base(g) for g in range(NG)], np.int64)
    col = gbase[egrp[eorder]] + ofs

    xs = np.zeros((ROWS, ECAP), _F32)
    xs[:D, col] = x[snd[eorder]].T
    xs[D, :] = 1.0
    rl = np.full(ECAP, float(RG), _F32)
    rl[col] = slot_of[rcv[eorder]]
    xs[D + 1] = rl
    xs[D + 2] = rl * rl

    slot_id = grp_of * RG + slot_of
    xr = np.zeros((SLOTS, D), _F32)
    xr[slot_id] = x[lo:lo + NLOC]
    # chunk-major layout for the device: [128, NCHUNK*D]
    xr_cm = np.ascontiguousarray(
        xr.reshape(NCHUNK, 128, D).transpose(1, 0, 2).reshape(128,
                                                              NCHUNK * D))

    tbl = np.zeros((ROWS, (NG + 2) * TBLW), _F32)
    t3 = tbl[:, :NG * TBLW].reshape(ROWS, NG, TBLW)
    t3[0:D, :, 0:D] = Wv[:, None, :]
    t3[D, :, D] = 1.0
    av = (A.T @ xr.T).astype(_F32)          # [D, SLOTS]
    t3[0:D, :, 65:TBLW] = av.reshape(D, NG, RG)
    r = np.arange(RG, dtype=_F32)
    t3[D, :, 65:TBLW] = -CMASK * r * r
    t3[D + 1, :, 65:TBLW] = 2.0 * CMASK * r
    t3[D + 2, :, 65:TBLW] = -CMASK

    return xs, tbl, xr_cm, slot_id


def kernel(x, edge_index, Wq, Wk, Wv, Wo, **_unused):
    x = np.asarray(x, _F32)
    edge_index = np.asarray(edge_index)
    Wq = np.asarray(Wq, _F32)
    Wk = np.asarray(Wk, _F32)
    Wv = np.asarray(Wv, _F32)
    Wo = np.asarray(Wo, _F32)
    sender = np.asarray(edge_index[0], np.int64)
    receiver = np.asarray(edge_index[1], np.int64)

    A = (Wq @ Wk.T).astype(_F32) * _F32(SCALE)

    nc = _get_nc()

    in_maps = []
    slot_ids = []
    t0 = time.time()
    for c in range(NCORES):
        xs, tbl, xr_cm, slot_id = _prep_core(x, sender, receiver, A, Wv, c)
        in_maps.append({"xs": xs, "tbl": tbl, "wo": Wo, "xloc": xr_cm})
        slot_ids.append(slot_id)
    print(f"[kernel] host prep: {time.time()-t0:.1f}s", flush=True)

    from concourse import bass_utils
    trace = bool(int(os.environ.get("KERNEL_TRACE", "0")))
    t0 = time.time()
    res = bass_utils.run_bass_kernel_spmd(
        nc, in_maps, core_ids=list(range(NCORES)), trace=trace)
    print(f"[kernel] device run: {time.time()-t0:.1f}s", flush=True)
    _CACHE["last_results"] = res

    out = np.empty((N, D), _F32)
    for c in range(NCORES):
        dev = res.results[c]["out"].reshape(128, NCHUNK, D)
        dev = dev.transpose(1, 0, 2).reshape(SLOTS, D)
        out[c * NLOC:(c + 1) * NLOC] = dev[slot_ids[c]]
    return out



# revision 3
# speedup vs baseline: 1.9844x; 1.9844x over previous
"""DenseFlashAttention (GNN segment-softmax attention) on 8 trn2 NeuronCores.

Sharding: receivers (and their incident edges) sharded across 8 cores;
DxD weights folded/replicated. Each core computes out rows for its 12500
receivers; host gathers.

Device algorithm per core (fp16 edge streams, f32 accumulation):
  - receivers bin-packed into 392 groups of 32: 72 "big" groups with 768
    edge slots (6 tiles), 320 "small" groups with 512 slots (4 tiles);
    per-group edge lists padded with self-masking pad edges (rl = 32).
  - xs [67, ECAP] fp16: rows 0..63 = x[sender]^T, 64 = 1, 65 = rl
    (receiver slot in group), 66 = rl^2. Zero-degree receivers get one
    dummy edge (x = 0, rl = slot) so denom >= 1 on device (no eps path).
  - per 128-edge tile t three PE ops:
      S  = xs_t^T @ atbl_g            [128, 32] f32 psum
           atbl rows 0..63 = a_r = (Wq Wk^T D^-0.5)^T x_r, rows 64..66
           encode -C (r - rl)^2 exactly in fp16 (C = 64) -> mask-in-matmul
      xt = transpose(xs_t[0:65])      [128, 65] fp16 psum (PE transpose)
      after 12-tile banks fill: ACT exp(S bank) -> p fp16 sbuf,
      DVE copy(xt bank) -> sbuf (2x 16-bit mode)
  - acc per tile: psO[65, 32] += xt^T @ p  (num rows 0..63 = sum p*x,
    row 64 = denom); Wv folded into the output proj (W2 = Wv @ Wo).
  - psO drains every 16 groups: ACT copy*2^-6 -> odn fp16 [65, SLOTS].
  - finals per 128-slot chunk: PE [odn^T @ [W2 | e64]] -> [128, 65] psum,
    DVE recip(col 64), DVE fused (num*rec + x) -> out fp16. Interleaved
    into the main loop at drain points (no serial tail).
"""

import os
import time
from contextlib import ExitStack

import numpy as np

# ---------------- static problem/config constants (hardcoded) ----------------
N = 100000
D = 64
E = 1600000
NCORES = 8
NLOC = N // NCORES            # 12500 receivers per core
RG = 32                       # receivers per group
NG = 392                      # groups per core
SLOTS = NG * RG               # 12544 receiver slots
NBIG = 72                     # 6-tile groups (cap 768 edges)
NSMALL = NG - NBIG            # 4-tile groups (cap 512 edges)
GT_BIG = 6
GT_SMALL = 4
GB_EDGE = GT_BIG * 128        # 768
GS_EDGE = GT_SMALL * 128      # 512
ECAP = NBIG * GB_EDGE + NSMALL * GS_EDGE   # 219136 edge slots per core
NTILES = NBIG * GT_BIG + NSMALL * GT_SMALL  # 1712 tiles per core
ROWS = 67                     # 64 x rows + ones + rl + rl^2
CMASK = 64.0                  # mask penalty coefficient
CH_G = 8                      # groups per DMA chunk
BNK = 12                      # tiles per psum bank (S + xt)
TST = 80                      # fp16 xt window stride in psum (160B, 32B-aligned)
PSO_G = 16                    # groups per psum-out bank
NCHUNK = SLOTS // 128         # 98 final projection chunks
SCALE = D ** -0.5
ODN_SC = 2.0 ** -6            # num/denom fp16 store scale
ACC_LAG = 2                   # banks between S/T emission and acc emission

_F32 = np.float32
_F16 = np.float16


def _group_base(g):
    return g * GB_EDGE if g < NBIG else \
        NBIG * GB_EDGE + (g - NBIG) * GS_EDGE


def _group_tiles(g):
    return GT_BIG if g < NBIG else GT_SMALL


# static per-tile map: tau -> (g, t_in_g, gt)
_TILE_INFO = []
for _g in range(NG):
    _gt = _group_tiles(_g)
    for _t in range(_gt):
        _TILE_INFO.append((_g, _t, _gt))
assert len(_TILE_INFO) == NTILES


# ---------------- device kernel (built/compiled once) ----------------
_CACHE = {}


def _build_nc():
    import concourse.tile as tile
    from concourse import bacc, masks, mybir

    f32 = mybir.dt.float32
    f16 = mybir.dt.float16
    nc = bacc.Bacc("TRN2", target_bir_lowering=False, debug=False,
                   num_devices=NCORES)
    xs_ap = nc.dram_tensor("xs", [ROWS, ECAP], f16,
                           kind="ExternalInput").ap()
    atbl_ap = nc.dram_tensor("atbl", [ROWS, NG * RG], f16,
                             kind="ExternalInput").ap()
    w2s_ap = nc.dram_tensor("w2s", [65, 65], f16, kind="ExternalInput").ap()
    xloc_ap = nc.dram_tensor("xloc", [128, NCHUNK * D], f16,
                             kind="ExternalInput").ap()
    out_ap = nc.dram_tensor("out", [128, NCHUNK * D], f16,
                            kind="ExternalOutput").ap()

    EXP = mybir.ActivationFunctionType.Exp
    COPY = mybir.ActivationFunctionType.Copy
    MUL = mybir.AluOpType.mult
    ADD = mybir.AluOpType.add

    with tile.TileContext(nc) as tc:
        with ExitStack() as octx:
            const_pool = octx.enter_context(tc.tile_pool(name="const", bufs=1))
            w2s_sb = const_pool.tile([65, 65], f16)
            nc.sync.dma_start(w2s_sb[:], w2s_ap[:, :])
            ident = const_pool.tile([65, 65], f16)
            masks.make_identity(nc, ident[:])
            # a-tables: first 16 groups up front (short pole), rest behind
            atbl_sb = const_pool.tile([ROWS, NG * RG], f16)
            nc.sync.dma_start(atbl_sb[:, :16 * RG], atbl_ap[:, :16 * RG])
            nc.sync.dma_start(atbl_sb[:, 16 * RG:], atbl_ap[:, 16 * RG:])
            # num^T (rows 0..63, *2^-6) + denom (row 64, *2^-6), fp16
            odn_sb = const_pool.tile([65, SLOTS], f16)

            stream_pool = octx.enter_context(
                tc.tile_pool(name="stream", bufs=4))
            xt_pool = octx.enter_context(tc.tile_pool(name="xt", bufs=4))
            p_pool = octx.enter_context(tc.tile_pool(name="p", bufs=4))
            fin_sb = octx.enter_context(tc.tile_pool(name="finsb", bufs=3))
            wk_sb = octx.enter_context(tc.tile_pool(name="wk", bufs=4))
            psS = octx.enter_context(
                tc.tile_pool(name="psS", bufs=2, space="PSUM"))
            psT = octx.enter_context(
                tc.tile_pool(name="psT", bufs=2, space="PSUM"))
            psO = octx.enter_context(
                tc.tile_pool(name="psO", bufs=2, space="PSUM"))
            fin_ps = octx.enter_context(
                tc.tile_pool(name="finps", bufs=2, space="PSUM"))

            # ---- final-chunk processing (deferred one drain for slack) ----
            def emit_finals(k0, nk):
                xl = fin_sb.tile([128, 4 * D], f16, tag="xl")
                nc.gpsimd.dma_start(xl[:, :nk * D],
                                    xloc_ap[:, k0 * D:(k0 + nk) * D])
                o8 = fin_sb.tile([128, 4 * D], f16, tag="o8")
                for j in range(nk):
                    k = k0 + j
                    fp = fin_ps.tile([128, 65], f32, tag="fp")
                    nc.tensor.matmul(
                        out=fp[:],
                        lhsT=odn_sb[:, k * 128:(k + 1) * 128],
                        rhs=w2s_sb[:], start=True, stop=True)
                    rec = wk_sb.tile([128, 1], f32, tag="rec")
                    nc.vector.reciprocal(rec[:], fp[:, 64:65])
                    nc.vector.scalar_tensor_tensor(
                        out=o8[:, j * D:(j + 1) * D],
                        in0=fp[:, 0:64], scalar=rec[:, 0:1],
                        in1=xl[:, j * D:(j + 1) * D],
                        op0=MUL, op1=ADD)
                nc.gpsimd.dma_start(
                    out_ap[:, k0 * D:(k0 + nk) * D], o8[:, :nk * D])

            # ---- acc + drain emission for one completed (p, xt) bank ----
            state = {"pso": None, "fin_q": []}

            def emit_acc(p_t, xt_t, tau0, nb):
                for ti in range(nb):
                    g, t, gt = _TILE_INFO[tau0 + ti]
                    pb, gi = divmod(g, PSO_G)
                    if gi == 0 and t == 0:
                        pso_t = psO.tile([65, PSO_G * RG], f32, tag="pso")
                        state["pso"] = pso_t
                    pso = state["pso"]
                    nc.tensor.matmul(
                        out=pso[:, gi * RG:(gi + 1) * RG],
                        lhsT=xt_t[:, ti * 65:(ti + 1) * 65],
                        rhs=p_t[:, ti * RG:(ti + 1) * RG],
                        start=(t == 0), stop=(t == gt - 1))
                    if t == gt - 1 and (gi == PSO_G - 1 or g == NG - 1):
                        cols = (gi + 1) * RG
                        nc.scalar.activation(
                            odn_sb[:, pb * PSO_G * RG:
                                   pb * PSO_G * RG + cols],
                            pso[:, :cols], COPY, scale=ODN_SC)
                        for kk in state["fin_q"]:
                            emit_finals(*kk)
                        state["fin_q"] = [
                            (pb * 4, min(4, NCHUNK - pb * 4))]

            # ---- main loop: chunks of groups -> tiles -> banks ----
            chunks = ([(0, GT_BIG, 1), (1, GT_BIG, CH_G - 1)] +
                      [(cg * CH_G, GT_BIG, CH_G)
                       for cg in range(1, NBIG // CH_G)] +
                      [(NBIG + cg * CH_G, GT_SMALL, CH_G)
                       for cg in range(NSMALL // CH_G)])
            tau = 0
            psS_cur = psT_cur = None
            bank_start = 0
            pending = []

            def finish_bank():
                nonlocal bank_start
                nb = tau - bank_start
                p_t = p_pool.tile([128, BNK * RG], f16, tag="p")
                nc.scalar.activation(
                    p_t[:, :nb * RG], psS_cur[:, :nb * RG], EXP)
                xt_t = xt_pool.tile([128, BNK * 65], f16, tag="xt")
                nc.vector.tensor_copy(
                    xt_t[:, :nb * 65].rearrange("p (t c) -> p t c", c=65),
                    psT_cur[:, :nb * TST].rearrange(
                        "p (t c) -> p t c", c=TST)[:, :, 0:65])
                pending.append((p_t, xt_t, bank_start, nb))
                bank_start = tau
                if len(pending) > ACC_LAG:
                    emit_acc(*pending.pop(0))

            for g0, gt, ng in chunks:
                gedge = gt * 128
                xs_t = stream_pool.tile([ROWS, CH_G * GB_EDGE], f16,
                                        tag="xs")
                c0 = _group_base(g0)
                nc.sync.dma_start(
                    xs_t[:, :ng * gedge], xs_ap[:, c0:c0 + ng * gedge])
                for gi in range(ng):
                    g = g0 + gi
                    for t in range(gt):
                        bi = tau - bank_start
                        if bi == 0:
                            psS_cur = psS.tile([128, BNK * RG], f32,
                                               tag="s")
                            psT_cur = psT.tile([128, BNK * TST], f16,
                                               tag="t")
                        col = gi * gedge + t * 128
                        nc.tensor.matmul(
                            out=psS_cur[:, bi * RG:(bi + 1) * RG],
                            lhsT=xs_t[:, col:col + 128],
                            rhs=atbl_sb[:, g * RG:(g + 1) * RG],
                            start=True, stop=True)
                        nc.tensor.transpose(
                            psT_cur[:, bi * TST:bi * TST + 65],
                            xs_t[0:65, col:col + 128], ident[:])
                        tau += 1
                        if tau - bank_start == BNK:
                            finish_bank()
            if tau > bank_start:
                finish_bank()
            while pending:
                emit_acc(*pending.pop(0))
            for kk in state["fin_q"]:
                emit_finals(*kk)

    nc.compile()
    return nc


def _get_nc():
    if "nc" not in _CACHE:
        t0 = time.time()
        _CACHE["nc"] = _build_nc()
        print(f"[kernel] bass trace+compile: {time.time()-t0:.1f}s",
              flush=True)
    return _CACHE["nc"]


# ---------------- host-side sharding / preprocessing ----------------

def _pack_groups(deg):
    """Assign receivers to groups: 72 big (<=768 edges) + 320 small
    (<=512), 32 receivers each. Returns (grp_of, slot_of)."""
    order = np.argsort(-deg, kind="stable")
    bin_of = np.empty(NLOC, np.int64)
    slot_of = np.empty(NLOC, np.int64)
    idx = np.arange(NLOC)
    bin_of[order] = idx % NG
    slot_of[order] = idx // NG
    bsum = np.bincount(bin_of, weights=deg.astype(np.float64), minlength=NG)
    # biggest-sum bins become the big class (groups 0..NBIG-1)
    rank = np.argsort(-bsum, kind="stable")
    perm = np.empty(NG, np.int64)
    perm[rank] = np.arange(NG)
    grp_of = perm[bin_of]
    gsum = np.bincount(grp_of, weights=deg.astype(np.float64), minlength=NG)

    members = [list(np.where(grp_of == g)[0]) for g in range(NG)]
    it = 0
    while True:
        small_over = [g for g in range(NBIG, NG) if gsum[g] > GS_EDGE]
        big_over = [g for g in range(NBIG) if gsum[g] > GB_EDGE]
        if not small_over and not big_over:
            break
        if small_over:
            gs = small_over[0]
            gb = int(np.argmin(gsum[:NBIG]))
            hs = max(members[gs], key=lambda r: deg[r])
            lb = min(members[gb], key=lambda r: deg[r])
        else:
            gb = big_over[0]
            gs = NBIG + int(np.argmin(gsum[NBIG:]))
            hs = min(members[gs], key=lambda r: deg[r])
            lb = max(members[gb], key=lambda r: deg[r])
        members[gs].remove(hs)
        members[gb].remove(lb)
        members[gs].append(lb)
        members[gb].append(hs)
        grp_of[hs], grp_of[lb] = gb, gs
        slot_of[hs], slot_of[lb] = slot_of[lb], slot_of[hs]
        gsum[gs] += deg[lb] - deg[hs]
        gsum[gb] += deg[hs] - deg[lb]
        it += 1
        assert it < 20000, "bin-pack repair failed"
    return grp_of, slot_of


def _prep_core(x_ext, sender, receiver, A, core):
    """Build xs/atbl/xloc fp16 arrays + slot map for one core."""
    lo = core * NLOC
    mask = (receiver >= lo) & (receiver < lo + NLOC)
    snd = sender[mask]
    rcv = receiver[mask] - lo

    deg = np.bincount(rcv, minlength=NLOC)
    empt = np.where(deg == 0)[0]
    if len(empt):
        # dummy self-edges (x = 0 row of x_ext) so denom >= 1 on device
        snd = np.concatenate([snd, np.full(len(empt), N, np.int64)])
        rcv = np.concatenate([rcv, empt])
        deg = np.bincount(rcv, minlength=NLOC)
    grp_of, slot_of = _pack_groups(deg)

    egrp = grp_of[rcv]
    eorder = np.argsort(egrp, kind="stable")
    cnt = np.bincount(egrp, minlength=NG)
    cum = np.concatenate([[0], np.cumsum(cnt)[:-1]])
    ofs = np.arange(len(eorder)) - np.repeat(cum, cnt)
    gbase = np.array([_group_base(g) for g in range(NG)], np.int64)
    col = gbase[egrp[eorder]] + ofs

    xs = np.zeros((ROWS, ECAP), _F16)
    xs[:D, col] = x_ext[snd[eorder]].T
    xs[D, :] = 1.0
    rl = np.full(ECAP, float(RG), _F32)
    rl[col] = slot_of[rcv[eorder]]
    xs[D + 1] = rl
    xs[D + 2] = rl * rl

    slot_id = grp_of * RG + slot_of
    xr = np.zeros((SLOTS, D), _F32)
    xr[slot_id] = x_ext[lo:lo + NLOC]
    # chunk-major layout for the device: [128, NCHUNK*D]
    xr_cm = np.ascontiguousarray(
        xr.reshape(NCHUNK, 128, D).transpose(1, 0, 2).reshape(
            128, NCHUNK * D)).astype(_F16)

    atbl = np.zeros((ROWS, NG * RG), _F32)
    a3 = atbl.reshape(ROWS, NG, RG)
    av = (A.T @ xr.T)                        # [D, SLOTS]
    a3[0:D] = av.reshape(D, NG, RG)
    r = np.arange(RG, dtype=_F32)
    a3[D] = -CMASK * r * r
    a3[D + 1] = 2.0 * CMASK * r
    a3[D + 2] = -CMASK

    return xs, atbl.astype(_F16), xr_cm, slot_id


def kernel(x, edge_index, Wq, Wk, Wv, Wo, **_unused):
    x = np.asarray(x, _F32)
    edge_index = np.asarray(edge_index)
    Wq = np.asarray(Wq, _F32)
    Wk = np.asarray(Wk, _F32)
    Wv = np.asarray(Wv, _F32)
    Wo = np.asarray(Wo, _F32)
    sender = np.asarray(edge_index[0], np.int64)
    receiver = np.asarray(edge_index[1], np.int64)

    A = (Wq @ Wk.T).astype(_F32) * _F32(SCALE)
    W2 = (Wv @ Wo).astype(_F32)
    w2s = np.zeros((65, 65), _F32)
    w2s[0:64, 0:64] = W2
    w2s[64, 64] = 1.0
    w2s = w2s.astype(_F16)
    x_ext = np.vstack([x, np.zeros((1, D), _F32)])   # row N = dummy sender

    nc = _get_nc()

    in_maps = []
    slot_ids = []
    t0 = time.time()
    for c in range(NCORES):
        xs, atbl, xr_cm, slot_id = _prep_core(x_ext, sender, receiver, A, c)
        in_maps.append({"xs": xs, "atbl": atbl, "w2s": w2s, "xloc": xr_cm})
        slot_ids.append(slot_id)
    print(f"[kernel] host prep: {time.time()-t0:.1f}s", flush=True)

    from concourse import bass_utils
    trace = bool(int(os.environ.get("KERNEL_TRACE", "0")))
    t0 = time.time()
    res = bass_utils.run_bass_kernel_spmd(
        nc, in_maps, core_ids=list(range(NCORES)), trace=trace)
    print(f"[kernel] device run: {time.time()-t0:.1f}s", flush=True)
    _CACHE["last_results"] = res

    out = np.empty((N, D), _F32)
    for c in range(NCORES):
        dev = res.results[c]["out"].astype(_F32).reshape(128, NCHUNK, D)
        dev = dev.transpose(1, 0, 2).reshape(SLOTS, D)
        out[c * NLOC:(c + 1) * NLOC] = dev[slot_ids[c]]
    return out
